# revision 19
# baseline (speedup 1.0000x reference)
"""Trainium2 Bass kernel for nn_FuzzyMultiLayer (K2 design).

Reference math (per point x in R^32, K=8 classes):
    L_k = tril(scale_k); z = L_k^{-1} (x - mu_k); maha_k = ||z||^2
    log_prob_k = -0.5*maha_k - 0.5*C*log(2pi) - log|det L_k|
    prob = exp(log_prob); g = prob * rsqrt(max(sum_k prob^2, 1e-12))
    out[.., k*C + c] = g_k * x_c
Since 0.5*C*log(2pi) = 29.44, prob_k <= ~9e-13 and sum prob^2 << 1e-12
always, so g_k = 1e6 * prob_k = exp(-0.5*maha_k + const_k) exactly
(const_k = log(1e6) - 0.5*C*log(2pi) - logdet_k).  No cross-class
normalization needed.

Sharding: pure data parallel, batch b -> core b.  Per-core x [65536, 32]
-> out [65536, 256].

K2 design notes (vs the v1 kernel this replaced):
  * PE per-instruction overhead (~60-120ns) and stream passes dominate,
    so the kernel minimizes PE instructions: NO x-transposes (host
    supplies x pre-transposed WITH a ones row -> the -v shift folds into
    the z matmul as a 33rd stationary row), NO per-class bias fixups,
    4 g-transposes + 20 matmuls per 2048 points.
  * All wide matmuls are 512-wide moving: z in f32r (x/Linv rounded at
    ~2^-13 -> ~5e-4 scale-relative output error, measured), mask-reduce
    in fp16 (u = z^2 in [0, 36], fp16 rounding adds ~1e-3).
  * maha for 2048 points accumulates into ONE [32, 512] psum bank
    (rows = 8q + k) via an 8-matmul accumulation group with zero-padded
    [128, 32] mask stationaries (out partition base must be 0 mod 32).
    One Exp (bias=const_k, scale=-0.5) covers all 2048 points.
  * Output is written bf16 (harness gate is 2e-2 scale-relative; bf16
    adds ~2e-3) halving write traffic; host casts back to fp32.
  * Host pre-permutes the point order (dram row n0+16p+s <-> point
    n0+128s+p) so every DMA is contiguous 1-8KB per partition; host
    un-permutes the output.

Per 2048-point iteration (32 iterations):
  DMA xt [33, 2048] f32r (channel-major x + ones row)
  DMA Xb [128, 512] bf16 (point-major x, rows (s c))
  8x  matmul z[q,cg] [128, 512] = ltv[:, cg].T @ xt[:, 512q:+512]
  8x  square u = z*z -> fp16 (split across ACT / Pool / DVE)
  8x  matmul maha [32, 512] += maskp[q,cg].T @ u   (one accum group)
  1x  ACT Exp g [32, 512] bf16 = exp(-0.5*maha + kc)
  4x  PE transpose g[:, 128a:+128] -> gt_ps[:, 32a:+32]  (bf16 streams)
  1x  copy gt_ps -> gtb bf16
  2x  outmul out[p, (q a k c)] = gtb[p, (a q k)] * Xb[p, (q a c)]
  DMA out [128, 4096] bf16
"""

import math
import os

import numpy as np
import ml_dtypes
from contextlib import ExitStack

import concourse.bacc as bacc
import concourse.tile as tile
from concourse import mybir
from concourse.bass_utils import run_bass_kernel_spmd

# Problem dims (hardcoded per contract)
B, H, W, C, K = 8, 256, 256, 32, 8
N = H * W          # points per core (one batch element per core)
N_CORES = 8
PTS = 2048         # points per macro-iteration
NIT = N // PTS     # 32 iterations
F32 = mybir.dt.float32
F32R = mybir.dt.float32r
FP16 = mybir.dt.float16
BF16 = mybir.dt.bfloat16

_BUILD_CACHE: dict = {}


def _sq_engine(nc, idx):
    """Engine rotation for the 8 squares per iteration (tunable).

    Squares read PSUM: GPSIMD can't access PSUM and DVE can't read two
    PSUM operands, so 'a' = ACT Square(z_psum) directly, and 'v' = DVE
    tensor_copy z->zb fp16 then DVE u = z_psum * zb_sbuf (the single
    fp16-rounded factor keeps the maha error at ~2^-12)."""
    pat = os.environ.get("FUZZY_SQ_PAT", "aaaa").replace(" ", "")
    ch = pat[idx % len(pat)]
    return {"a": "act", "v": "dve"}[ch]


def _build_nc():
    nc = bacc.Bacc("TRN2", target_bir_lowering=False, debug=False,
                   num_devices=N_CORES)

    xt_in = nc.dram_tensor("xt", [98, N], FP16, kind="ExternalInput").ap()
    zpad_in = nc.dram_tensor("zpad", [32, 2048], FP16, kind="ExternalInput").ap()
    xb_in = nc.dram_tensor("xb", [N, C], BF16, kind="ExternalInput").ap()
    ltv_in = nc.dram_tensor("ltv", [128, 256], FP16, kind="ExternalInput").ap()
    UDT = getattr(mybir.dt, os.environ.get("FUZZY_UDT", "float16"))
    maskp_in = nc.dram_tensor("maskp", [128, 256], UDT, kind="ExternalInput").ap()
    kc_in = nc.dram_tensor("kc32", [32, 1], F32, kind="ExternalInput").ap()
    id_in = nc.dram_tensor("id32", [32, 32], BF16, kind="ExternalInput").ap()
    out_dram = nc.dram_tensor("out", [N, K * C], BF16, kind="ExternalOutput").ap()


    with tile.TileContext(nc, pool_alloc_mode="queue") as tc, ExitStack() as ctx:
        const = ctx.enter_context(tc.tile_pool(name="const", bufs=1))
        ltv_sb = const.tile([128, 256], FP16)
        nc.sync.dma_start(ltv_sb[:], ltv_in[:])
        maskp_sb = const.tile([128, 256], UDT)
        nc.sync.dma_start(maskp_sb[:], maskp_in[:])
        kc_sb = const.tile([32, 1], F32)
        nc.sync.dma_start(kc_sb[:], kc_in[:])
        id_sb = const.tile([32, 32], BF16)
        nc.sync.dma_start(id_sb[:], id_in[:])

        xt_tiles = [const.tile([128, PTS], FP16, name=f"xtb{i}") for i in range(2)]
        for t in xt_tiles:
            # zero the pad rows once; stationary pad rows are zero anyway
            nc.sync.dma_start(t[98:128, :], zpad_in[0:30, :])
        zb_pool = ctx.enter_context(tc.tile_pool(name="zb", bufs=3))
        xb_pool = ctx.enter_context(tc.tile_pool(name="xb", bufs=3))
        z_pool = ctx.enter_context(tc.tile_pool(name="z_ps", bufs=3, space="PSUM"))
        u_pool = ctx.enter_context(tc.tile_pool(name="u_sb", bufs=8))
        g_pool = ctx.enter_context(tc.tile_pool(name="g_sb", bufs=3))
        gt_pool = ctx.enter_context(tc.tile_pool(name="gt_ps", bufs=1, space="PSUM"))
        gtb_pool = ctx.enter_context(tc.tile_pool(name="gtb", bufs=3))
        out_pool = ctx.enter_context(tc.tile_pool(name="out_sb", bufs=4))

        # maha double-buffer inside ONE psum bank: row halves alternate per
        # iteration (matmul out partition base must be 0 mod 32).
        mahac = ctx.enter_context(tc.tile_pool(name="mahac", bufs=1, space="PSUM"))
        mahabuf = mahac.tile([64, 512], F32, name="mahabuf")

        def emit_head(it):
            """DMAs + z matmuls + squares + maha accumulation for iter it."""
            n0 = it * PTS
            xt = xt_tiles[it % 2]
            nc.sync.dma_start(xt[0:98, :], xt_in[:, n0:n0 + PTS])
            Xb = xb_pool.tile([128, 512], BF16)
            nc.sync.dma_start(
                Xb[:], xb_in[n0:n0 + PTS, :].rearrange("(p s) c -> p (s c)", s=16))

            maha = mahabuf[32 * (it % 2):32 * (it % 2) + 32, :]
            for q in range(4):
                z = z_pool.tile([128, 1024], F32)  # two psum banks: cg0 | cg1
                for cg in range(2):
                    nc.tensor.matmul(
                        z[:, 512 * cg:512 * (cg + 1)],
                        ltv_sb[:, 128 * cg:128 * (cg + 1)],
                        xt[0:128, 512 * q:512 * (q + 1)], start=True, stop=True)
                u = u_pool.tile([128, 1024], UDT)
                eng = _sq_engine(nc, q)
                if eng == "act":
                    nc.scalar.activation(
                        u[:], z[:], mybir.ActivationFunctionType.Square)
                else:
                    zb = zb_pool.tile([128, 1024], FP16)
                    nc.vector.tensor_copy(zb[:], z[:])
                    nc.vector.tensor_mul(u[:], z[:], zb[:])
                for cg in range(2):
                    nc.tensor.matmul(
                        maha, maskp_sb[:, 32 * (2 * q + cg):32 * (2 * q + cg + 1)],
                        u[:, 512 * cg:512 * (cg + 1)],
                        start=(q == 0 and cg == 0), stop=(q == 3 and cg == 1))
            return Xb

        def emit_tail(it, Xb):
            """Exp + g transposes + outmul + out DMA for iter it."""
            n0 = it * PTS
            maha = mahabuf[32 * (it % 2):32 * (it % 2) + 32, :]
            g = g_pool.tile([32, 512], BF16)
            nc.scalar.activation(
                g[:], maha, mybir.ActivationFunctionType.Exp,
                bias=kc_sb[:], scale=-0.5)

            gt_ps = gt_pool.tile([128, 128], BF16)
            for a in range(4):
                nc.tensor.transpose(
                    gt_ps[:, 32 * a:32 * (a + 1)], g[:, 128 * a:128 * (a + 1)],
                    id_sb[:])
            gtb = gtb_pool.tile([128, 128], BF16)
            nc.vector.tensor_copy(gtb[:], gt_ps[:])

            out_sb = out_pool.tile([128, 4096], BF16)
            # Chunk order s = 4a + q (the g-transposes' native column order):
            # out[p, (s k c)] = gtb[p, (s k)] * Xb[p, (s c)].  Host indexes
            # xb/out rows with the same s so all APs stay 3-free-dim.
            o_ap = out_sb[:].rearrange("p (s k c) -> p s k c", s=16, k=K)
            g_ap = (gtb[:].rearrange("p (s k) -> p s k", s=16)
                    .unsqueeze(3).broadcast_to([128, 16, K, C]))
            x_ap = (Xb[:].rearrange("p (s c) -> p s c", s=16)
                    .unsqueeze(2).broadcast_to([128, 16, K, C]))
            sv = int(os.environ.get("FUZZY_OSPLIT", "16"))  # s-rows on DVE
            if sv > 0:
                nc.vector.tensor_mul(o_ap[:, 0:sv], g_ap[:, 0:sv], x_ap[:, 0:sv])
            if sv < 16:
                nc.gpsimd.tensor_mul(o_ap[:, sv:16], g_ap[:, sv:16], x_ap[:, sv:16])
            dst = out_dram[n0:n0 + PTS, :].rearrange("(p s) c -> p (s c)", s=16)
            nc.gpsimd.dma_start(dst, out_sb[:])

        # Software-pipelined emission: iteration i's tail (Exp..out-DMA) is
        # emitted AFTER iteration i+1's head, so the in-order ACT queue never
        # stalls next-iteration squares behind an Exp waiting on PE.
        prev = None
        for it in range(NIT):
            Xb = emit_head(it)
            if prev is not None:
                emit_tail(it - 1, prev)
            prev = Xb
        emit_tail(NIT - 1, prev)

    nc.compile()
    return nc


def _host_constants(mean: np.ndarray, scale: np.ndarray):
    """Tiny per-class parameter transforms, done in fp64 on host."""
    L = np.tril(scale.astype(np.float64))                       # [K, C, C]
    eye = np.eye(C, dtype=np.float64)
    Linv = np.stack([np.linalg.solve(L[k], eye) for k in range(K)])  # [K, C, C]
    v = np.einsum("kcd,kd->kc", Linv, mean.astype(np.float64))  # [K, C]
    logdet = np.log(np.abs(np.diagonal(L, axis1=-2, axis2=-1))).sum(-1)  # [K]
    kconst = math.log(1e6) - 0.5 * C * math.log(2.0 * math.pi) - logdet  # [K]

    # Split-fp16 z: one contract-98 fp16 matmul computes L x - v with
    # compensation:  z = Lh xh + (-vh) + Lh xl + Ll xh + (-vl)
    #   rows 0-31: xh (stat Lh)   row 32: ones (stat -vh)
    #   rows 33-64: xl (stat Lh)  rows 65-96: xh (stat Ll)
    #   row 97: ones (stat -vl)   rows 98-127: zero pad
    ltv = np.zeros((128, 256), dtype=np.float16)
    for k in range(K):
        cg, kk = divmod(k, 4)
        col0 = 128 * cg + 32 * kk
        LT = Linv[k].T.astype(np.float64)            # [c, cc]
        Lh = LT.astype(np.float16)
        Ll = (LT - Lh.astype(np.float64)).astype(np.float16)
        vh = (-v[k]).astype(np.float16)
        vl = (-v[k] - vh.astype(np.float64)).astype(np.float16)
        ltv[0:32, col0:col0 + 32] = Lh
        ltv[32, col0:col0 + 32] = vh
        ltv[33:65, col0:col0 + 32] = Lh
        ltv[65:97, col0:col0 + 32] = Ll
        ltv[97, col0:col0 + 32] = vl
    # maskp[32kk+cc, 32*(2q+cg) + (8q + 4cg + kk)] = 1
    maskp = np.zeros((128, 256), dtype=np.float16)
    for q in range(4):
        for cg in range(2):
            for kk in range(4):
                maskp[32 * kk:32 * (kk + 1),
                      32 * (2 * q + cg) + 8 * q + 4 * cg + kk] = 1.0
    # kc32[8q + k] = kconst_k
    kc32 = np.tile(kconst.astype(np.float32), 4).reshape(32, 1)
    id32 = np.eye(32, dtype=ml_dtypes.bfloat16)
    return {"ltv": ltv, "maskp": maskp, "kc32": kc32, "id32": id32}


def kernel(x: np.ndarray, mean: np.ndarray, scale: np.ndarray,
           _trace: bool = False) -> np.ndarray:
    x = np.asarray(x, dtype=np.float32)
    mean = np.asarray(mean, dtype=np.float32)
    scale = np.asarray(scale, dtype=np.float32)
    assert x.shape == (B, H, W, C)
    key = "nc_k2"
    if key not in _BUILD_CACHE:
        _BUILD_CACHE[key] = _build_nc()
    nc = _BUILD_CACHE[key]

    consts = _host_constants(mean, scale)
    in_maps = []
    for b in range(N_CORES):
        xb_flat = x[b].reshape(N, C)
        xT = xb_flat.T.astype(np.float64)
        xh = xT.astype(np.float16)
        xl = (xT - xh.astype(np.float64)).astype(np.float16)
        xt = np.empty((98, N), dtype=np.float16)
        xt[0:32] = xh
        xt[32] = 1.0
        xt[33:65] = xl
        xt[65:97] = xh
        xt[97] = 1.0
        # permuted point-major bf16: dram row 2048*it + 16p + s <-> point
        # 2048*it + 128s + p
        # dram row 2048it + 16p + (4a+q)  <->  point 2048it + 512q + 128a + p
        xbp = np.ascontiguousarray(
            xb_flat.reshape(NIT, 4, 4, 128, C).transpose(0, 3, 2, 1, 4)
        ).reshape(N, C).astype(ml_dtypes.bfloat16)
        m = {"xt": xt, "xb": xbp,
             "zpad": np.zeros((32, 2048), np.float16)}
        m.update(consts)
        in_maps.append(m)

    res = run_bass_kernel_spmd(nc, in_maps, list(range(N_CORES)), trace=_trace)
    if _trace:
        _BUILD_CACHE["last_exec_time_ns"] = res.exec_time_ns
        _BUILD_CACHE["last_profile"] = res.profile_json
    outs = []
    for b in range(N_CORES):
        o = np.asarray(res.results[b]["out"])           # [N, 256] bf16, permuted
        o = o.reshape(NIT, 128, 4, 4, K * C).transpose(0, 3, 2, 1, 4)
        outs.append(o.reshape(H, W, K * C).astype(np.float32))
    return np.stack(outs)
